# revision 8
# baseline (speedup 1.0000x reference)
"""GAT attention kernel for 8 trn2 NeuronCores (Bass/Tile), bf16 q-layout,
paired-tile pipeline.

Math (restructured from the reference):
    wa1 = W @ a1, wa2 = W @ a2                      (host, weight folding)
    x'  = x * wa2[f]   (host)                        so  sj[n,k] = sum_f x'[n,k,f]
    x0w1 = x0 * wa1[f] (host)                        so  si[n]   = sum_f x0w1[n,f]
    x0' = x0 * wa2[f]  (host),  W' = W / wa2[:,None] (host)
    s       = si + sj
    p       = exp(prelu(s, 0.2)) * adj
    Z'      = sum_k p + 16*EPS                       (per node)
    U       = sum_k (p_k+EPS) * x'_k  +  Z' * x0'    (unnormalized, PSUM)
    out     = elu((U @ W') / Z')                     (/Z' via ACT per-partition scale)
    elu(z)  = relu(z) - relu(1 - exp(z))

Sharding: node dim N padded 50000 -> 51200 = 8 cores * 25 pairs * 256 rows.
Each round processes a PAIR of 128-row tiles from one 9280B/partition DMA;
small ops (score TT/reduce chain, softmax, activations) run once per pair,
halving fixed per-instruction overheads.

Per 128-row tile the 2048 (n,k) pairs form 16 blocks of [128 q, 128 f] bf16
(q = 16*(n%8) + k, block b = n_tile//8). Scores for the whole pair (32 sj
segments + 2 si segments) come from 2 halving TTs + ONE segmented
tensor_reduce on DVE.
"""

import numpy as np
import ml_dtypes

N, K, F = 50000, 16, 128
ALPHA = 0.2
NCORES = 8
TILE = 128
NTILES = 50                  # padded (6400 rows/core, 6272 real)
NPAIRS = NTILES // 2
RPC_REAL = 6272              # real rows per core
BPT = K                      # nk-blocks per tile = 16
XCOLS = BPT * F + F + F + K  # per-tile bf16 cols = 2320
XC2 = 2 * XCOLS              # 4640
# pair-tile column map
OFF_XB = 2 * BPT * F         # 4096: end of the two x' block regions
OFF_X0W1 = 4096              # x0w1 A | x0w1 B  (score segments 32, 33)
OFF_X0P = 4352               # x0' A | x0' B
OFF_ADJ = 4608               # adj A | adj B (s-layout)
EPS = 1e-12

BF16 = ml_dtypes.bfloat16

_NC_CACHE = {}


def _consts_np():
    p = np.arange(128)
    j8 = np.arange(8)
    b16 = np.arange(16)
    ident = np.eye(128, dtype=np.float32)
    Cm = (p[:, None] % 8 == p[None, :] // 16).astype(np.float32)
    segbig = (p[:, None] // 16 == (p[None, :] % 8)).astype(np.float32)
    seg = (p[:, None] // 16 == j8[None, :]).astype(np.float32)
    seg8 = (p[:, None] // 8 == b16[None, :]).astype(np.float32)
    return ident, Cm, segbig, seg, seg8


def _consts_full_np(W, a):
    W = np.asarray(W, np.float64)
    a = np.asarray(a, np.float64)
    wa1 = W @ a[:F, 0]
    wa2 = W @ a[F:, 0]
    Wp = W / wa2[:, None]
    ident, Cm, segbig, seg, seg8 = _consts_np()
    cst = np.concatenate(
        [segbig, segbig, Cm, ident, Wp.astype(np.float32), seg8, seg], axis=1)
    return np.ascontiguousarray(cst).astype(BF16), wa1, wa2  # [128, 664]


def _build_nc(npairs=NPAIRS, finalize=True):
    import concourse.mybir as mybir
    import concourse.tile as tile
    from concourse import bacc

    fp = mybir.dt.float32
    bf = mybir.dt.bfloat16
    AF = mybir.ActivationFunctionType
    OP = mybir.AluOpType
    AX = mybir.AxisListType

    nc = bacc.Bacc("TRN2")
    xd = nc.dram_tensor("xd", [npairs, 128, XC2], bf, kind="ExternalInput")
    cst = nc.dram_tensor("cst", [128, 664], bf, kind="ExternalInput")
    yd = nc.dram_tensor("yd", [2 * npairs, 128, F], bf, kind="ExternalOutput")

    with tile.TileContext(nc) as tc:
        with (
            tc.tile_pool(name="const", bufs=1) as constp,
            tc.tile_pool(name="xin", bufs=7) as xin,
            tc.tile_pool(name="sm", bufs=5) as sm,
            tc.tile_pool(name="med", bufs=3) as med,
            tc.tile_pool(name="big", bufs=3) as big,
            tc.tile_pool(name="yout", bufs=3) as yout,
            tc.tile_pool(name="ps", bufs=1, space="PSUM") as ps,
        ):
            consts = constp.tile([128, 664], bf)
            nc.sync.dma_start(out=consts, in_=cst[:, :])
            SEGBIG2 = consts[:, 0:256]
            Cm = consts[:, 256:384]
            IDENT = consts[:, 384:512]
            Wp = consts[:, 512:640]
            SEG8 = consts[:, 640:656]
            SEG = consts[:, 656:664]

            st = {}

            # Phases are emitted oldest-tile-first each round so every
            # in-order engine drains without same-round cross-engine waits:
            # each op's inputs come from a PREVIOUS round (except the
            # softmax hop chain, which tails the round by design).

            def phase_load(t):          # round t
                xall = xin.tile([128, XC2], bf, tag="x")
                nc.sync.dma_start(out=xall, in_=xd[t])
                st[t] = {"xall": xall}

            def phase_score(t):         # round t+2: pure DVE
                d = st[t]
                xall = d["xall"]
                # 34-segment score sums (32 sj blocks + 2 si rows)
                sv = xall[:, 0:34 * F].rearrange("p (s f) -> p s f", f=F)
                h1 = med.tile([128, 34 * 64], bf, tag="h1")
                nc.vector.tensor_tensor(
                    out=h1.rearrange("p (s f) -> p s f", f=64),
                    in0=sv[:, :, 0:64], in1=sv[:, :, 64:128], op=OP.add)
                h1v = h1.rearrange("p (s f) -> p s f", f=64)
                h2 = med.tile([128, 34 * 32], bf, tag="h2")
                nc.vector.tensor_tensor(
                    out=h2.rearrange("p (s f) -> p s f", f=32),
                    in0=h1v[:, :, 0:32], in1=h1v[:, :, 32:64], op=OP.add)
                s17 = sm.tile([128, 34], fp, tag="s17")
                nc.vector.tensor_reduce(
                    out=s17, in_=h2.rearrange("p (s f) -> p s f", f=32),
                    axis=AX.X, op=OP.add)
                d["s17"] = s17

            def phase_softmax(t):       # round t+3: the hop chain (tails)
                d = st[t]
                xall = d["xall"]
                s17 = d["s17"]
                # scatter si (natural [n,1]) into s-layout via SEG8 + Cm
                Dt2 = sm.tile([128, 32], bf, tag="Dt")
                nc.gpsimd.tensor_scalar_mul(out=Dt2[:, 0:16], in0=SEG8,
                                            scalar1=s17[:, 32:33])
                nc.gpsimd.tensor_scalar_mul(out=Dt2[:, 16:32], in0=SEG8,
                                            scalar1=s17[:, 33:34])
                si_ps = ps.tile([128, 32], fp, tag="si", bufs=2)
                nc.tensor.matmul(si_ps, lhsT=Cm, rhs=Dt2, start=True, stop=True)
                s2 = sm.tile([128, 32], fp, tag="s2")
                nc.vector.scalar_tensor_tensor(
                    out=s2, in0=s17[:, 0:32], scalar=0.0, in1=si_ps,
                    op0=OP.add, op1=OP.add)
                ls = sm.tile([128, 32], fp, tag="ls")
                nc.scalar.activation(out=ls, in_=s2, func=AF.Prelu, alpha=ALPHA)
                exp_s = sm.tile([128, 32], bf, tag="exp_s")
                nc.scalar.activation(out=exp_s, in_=ls, func=AF.Exp)
                p_s = sm.tile([128, 32], bf, tag="p_s")
                nc.gpsimd.tensor_mul(out=p_s, in0=exp_s,
                                     in1=xall[:, OFF_ADJ:OFF_ADJ + 32])
                ZT_ps = ps.tile([32, 8], fp, tag="zt", bufs=2)
                nc.tensor.matmul(ZT_ps, lhsT=p_s, rhs=SEG,
                                 start=True, stop=True)
                d["p_s"] = p_s
                d["ZT_ps"] = ZT_ps

            def phase_z(t):             # round t+4: tz + Z scatter
                d = st[t]
                tz = sm.tile([32, 8], fp, tag="tz")
                nc.scalar.activation(out=tz, in_=d["ZT_ps"], func=AF.Copy,
                                     bias=16.0 * EPS)
                zn = sm.tile([128, 2], fp, tag="zn")
                nc.sync.dma_start(out=zn[:, 0:1], in_=tz[0:16, :])
                nc.sync.dma_start(out=zn[:, 1:2], in_=tz[16:32, :])
                d["zn"] = zn

            def phase_xbar(t):          # round t+5
                d = st[t]
                xall = d["xall"]
                zn = d["zn"]
                rz = sm.tile([128, 2], fp, tag="rz")
                nc.vector.reciprocal_approx_fast(rz, zn)
                d["rz"] = rz
                x0z = big.tile([128, 256], bf, tag="x0z")
                for h in (0, 1):
                    nc.vector.tensor_scalar(
                        out=x0z[:, 128 * h:128 * h + 128],
                        in0=xall[:, OFF_X0P + 128 * h:OFF_X0P + 128 * h + 128],
                        scalar1=zn[:, h:h + 1], scalar2=None, op0=OP.mult)
                attsegU = big.tile([128, 256], bf, tag="attsegU")
                p_bc = d["p_s"].rearrange("p (b o) -> p b o", o=1)
                nc.vector.scalar_tensor_tensor(
                    out=attsegU.rearrange("p (b j) -> p b j", j=8),
                    in0=p_bc.to_broadcast([128, 32, 8]), scalar=EPS,
                    in1=SEGBIG2.rearrange("p (b j) -> p b j", j=8),
                    op0=OP.add, op1=OP.mult)
                xbarT_ps = ps.tile([128, 256], fp, tag="xb", bufs=2)
                for h in (0, 1):
                    co = 128 * h
                    nc.tensor.matmul(
                        xbarT_ps[:, co:co + 128],
                        lhsT=x0z[:, co:co + 128], rhs=IDENT,
                        start=True, stop=False, skip_group_check=True)
                    for b in range(BPT):
                        nc.tensor.matmul(
                            xbarT_ps[:, co + 8 * b:co + 8 * b + 8],
                            lhsT=xall[:, 2048 * h + b * F:2048 * h + (b + 1) * F],
                            rhs=attsegU[:, co + 8 * b:co + 8 * b + 8],
                            start=False, stop=(b == BPT - 1),
                            skip_group_check=True)
                d["xbarT_ps"] = xbarT_ps

            def phase_fin(t):           # round t+6: ST + final GEMM
                d = st[t]
                ST = big.tile([128, 256], bf, tag="ST")
                nc.scalar.activation(out=ST, in_=d["xbarT_ps"], func=AF.Copy)
                Y_ps = ps.tile([128, 256], fp, tag="yy", bufs=2)
                for h in (0, 1):
                    co = 128 * h
                    nc.tensor.matmul(Y_ps[:, co:co + 128],
                                     lhsT=ST[:, co:co + 128], rhs=Wp,
                                     start=True, stop=True)
                d["Y_ps"] = Y_ps

            def phase_act(t):           # round t+7: elu activations
                d = st[t]
                rz = d["rz"]
                Y_ps = d["Y_ps"]
                e = yout.tile([128, 256], bf, tag="e")
                r = yout.tile([128, 256], bf, tag="r")
                for h in (0, 1):
                    co = 128 * h
                    nc.scalar.activation(out=e[:, co:co + 128],
                                         in_=Y_ps[:, co:co + 128],
                                         func=AF.Exp, scale=rz[:, h:h + 1])
                    nc.scalar.activation(out=r[:, co:co + 128],
                                         in_=Y_ps[:, co:co + 128],
                                         func=AF.Relu, scale=rz[:, h:h + 1])
                v = yout.tile([128, 256], bf, tag="v")
                nc.scalar.activation(out=v, in_=e, func=AF.Relu,
                                     scale=-1.0, bias=1.0)
                d["r"] = r
                d["v"] = v

            def phase_out(t):           # round t+8: y + store
                d = st[t]
                y = yout.tile([128, 256], bf, tag="y")
                nc.vector.tensor_tensor(out=y, in0=d["r"], in1=d["v"],
                                        op=OP.subtract)
                nc.sync.dma_start(out=yd[2 * t], in_=y[:, 0:128])
                nc.sync.dma_start(out=yd[2 * t + 1], in_=y[:, 128:256])
                del st[t]

            for r in range(npairs + 8):
                if 0 <= r - 8 < npairs:
                    phase_out(r - 8)
                if 0 <= r - 7 < npairs:
                    phase_act(r - 7)
                if 0 <= r - 6 < npairs:
                    phase_fin(r - 6)
                if 0 <= r - 5 < npairs:
                    phase_xbar(r - 5)
                if 0 <= r - 4 < npairs:
                    phase_z(r - 4)
                if 0 <= r - 2 < npairs:
                    phase_score(r - 2)
                if 0 <= r - 3 < npairs:
                    phase_softmax(r - 3)
                if r < npairs:
                    phase_load(r)

    if finalize:
        nc.finalize()
    return nc


def _get_nc(npairs=NPAIRS):
    if npairs not in _NC_CACHE:
        _NC_CACHE[npairs] = _build_nc(npairs)
    return _NC_CACHE[npairs]


def _shard_inputs(orignal_x, x, adj, W, a, ncores=NCORES, ntiles=NTILES):
    f32 = np.float32
    rpc = TILE * ntiles          # padded rows per core (6400)
    x = np.asarray(x, f32)
    x0 = np.asarray(orignal_x, f32)
    adj = np.asarray(adj, np.int32)
    cst, wa1, wa2 = _consts_full_np(W, a)
    wa1_f = wa1.astype(f32)
    wa2_f = wa2.astype(f32)
    n = x.shape[0]
    assert n <= RPC_REAL * ncores

    in_maps = []
    for c in range(ncores):
        lo = c * RPC_REAL
        hi = min((c + 1) * RPC_REAL, n)
        rows = hi - lo
        xc = x[lo:hi]
        x0c = x0[lo:hi]
        adjc = adj[lo:hi]
        if rows < rpc:
            pad = rpc - rows
            xc = np.concatenate([xc, np.zeros((pad, K, F), f32)])
            x0c = np.concatenate([x0c, np.zeros((pad, F), f32)])
            adjc = np.concatenate([adjc, np.zeros((pad, K), np.int32)])
        # per-tile packs [50, 128, *]
        xp = (xc * wa2_f[None, None, :]).astype(BF16)
        xp50 = xp.reshape(ntiles, 16, 8, K, F).transpose(
            0, 2, 3, 1, 4).reshape(ntiles, 128, BPT * F)
        x0w1 = (x0c * wa1_f[None, :]).astype(BF16).reshape(ntiles, 128, F)
        x0p = (x0c * wa2_f[None, :]).astype(BF16).reshape(ntiles, 128, F)
        adj50 = adjc.astype(BF16).reshape(ntiles, 16, 8, K).transpose(
            0, 2, 3, 1).reshape(ntiles, 128, K)
        xdev = np.empty((NPAIRS, 128, XC2), BF16)
        xdev[:, :, 0:2048] = xp50[0::2]
        xdev[:, :, 2048:4096] = xp50[1::2]
        xdev[:, :, OFF_X0W1:OFF_X0W1 + F] = x0w1[0::2]
        xdev[:, :, OFF_X0W1 + F:OFF_X0W1 + 2 * F] = x0w1[1::2]
        xdev[:, :, OFF_X0P:OFF_X0P + F] = x0p[0::2]
        xdev[:, :, OFF_X0P + F:OFF_X0P + 2 * F] = x0p[1::2]
        xdev[:, :, OFF_ADJ:OFF_ADJ + K] = adj50[0::2]
        xdev[:, :, OFF_ADJ + K:OFF_ADJ + 2 * K] = adj50[1::2]
        in_maps.append({"xd": xdev, "cst": cst})
    return in_maps


_LAST_RESULTS = None


def kernel(orignal_x, x, adj, W, a):
    import os
    os.environ.setdefault("JAX_PLATFORMS", "")
    from concourse.bass_utils import run_bass_kernel_spmd

    global _LAST_RESULTS
    nc = _get_nc()
    in_maps = _shard_inputs(orignal_x, x, adj, W, a)
    res = run_bass_kernel_spmd(nc, in_maps, list(range(NCORES)))
    _LAST_RESULTS = res
    y = np.concatenate(
        [np.asarray(r["yd"]).astype(np.float32).reshape(
            TILE * NTILES, F)[:RPC_REAL]
         for r in res.results], axis=0)
    return np.ascontiguousarray(y[:N])


# revision 10
# speedup vs baseline: 1.0298x; 1.0298x over previous
"""GAT attention kernel for 8 trn2 NeuronCores (Bass/Tile), bf16 q-layout,
paired-tile pipeline.

Math (restructured from the reference):
    wa1 = W @ a1, wa2 = W @ a2                      (host, weight folding)
    x'  = x * wa2[f]   (host)                        so  sj[n,k] = sum_f x'[n,k,f]
    x0w1 = x0 * wa1[f] (host)                        so  si[n]   = sum_f x0w1[n,f]
    x0' = x0 * wa2[f]  (host),  W' = W / wa2[:,None] (host)
    s       = si + sj
    p       = exp(prelu(s, 0.2)) * adj
    Z'      = sum_k p + 16*EPS                       (per node)
    U       = sum_k (p_k+EPS) * x'_k  +  Z' * x0'    (unnormalized, PSUM)
    out     = elu((U @ W') / Z')                     (/Z' via ACT per-partition scale)
    elu(z)  = relu(z) - relu(1 - exp(z))

Sharding: node dim N padded 50000 -> 51200 = 8 cores * 25 pairs * 256 rows.
Each round processes a PAIR of 128-row tiles from one 9280B/partition DMA;
small ops (score TT/reduce chain, softmax, activations) run once per pair,
halving fixed per-instruction overheads.

Per 128-row tile the 2048 (n,k) pairs form 16 blocks of [128 q, 128 f] bf16
(q = 16*(n%8) + k, block b = n_tile//8). Scores for the whole pair (32 sj
segments + 2 si segments) come from 2 halving TTs + ONE segmented
tensor_reduce on DVE.
"""

import numpy as np
import ml_dtypes

N, K, F = 50000, 16, 128
ALPHA = 0.2
NCORES = 8
TILE = 128
NTILES = 50                  # padded (6400 rows/core, 6272 real)
NPAIRS = NTILES // 2
RPC_REAL = 6272              # real rows per core
BPT = K                      # nk-blocks per tile = 16
XCOLS = BPT * F + F + F + K  # per-tile bf16 cols = 2320
XC2 = 2 * XCOLS              # 4640
# pair-tile column map
OFF_XB = 2 * BPT * F         # 4096: end of the two x' block regions
OFF_X0W1 = 4096              # x0w1 A | x0w1 B  (score segments 32, 33)
OFF_X0P = 4352               # x0' A | x0' B
OFF_ADJ = 4608               # adj A | adj B (s-layout)
EPS = 1e-12

BF16 = ml_dtypes.bfloat16

_NC_CACHE = {}


def _consts_np():
    p = np.arange(128)
    j8 = np.arange(8)
    b16 = np.arange(16)
    ident = np.eye(128, dtype=np.float32)
    Cm = (p[:, None] % 8 == p[None, :] // 16).astype(np.float32)
    segbig = (p[:, None] // 16 == (p[None, :] % 8)).astype(np.float32)
    seg = (p[:, None] // 16 == j8[None, :]).astype(np.float32)
    seg8 = (p[:, None] // 8 == b16[None, :]).astype(np.float32)
    return ident, Cm, segbig, seg, seg8


def _consts_full_np(W, a):
    W = np.asarray(W, np.float64)
    a = np.asarray(a, np.float64)
    wa1 = W @ a[:F, 0]
    wa2 = W @ a[F:, 0]
    Wp = W / wa2[:, None]
    ident, Cm, segbig, seg, seg8 = _consts_np()
    ones32 = np.ones((128, 32), dtype=np.float32)
    epsc = np.full((128, 8), 16.0 * EPS / 128.0, dtype=np.float32)
    cst = np.concatenate(
        [segbig, segbig, Cm, ident, Wp.astype(np.float32), seg8, seg,
         ones32, epsc], axis=1)
    return np.ascontiguousarray(cst).astype(BF16), wa1, wa2  # [128, 704]


def _build_nc(npairs=NPAIRS, finalize=True):
    import concourse.mybir as mybir
    import concourse.tile as tile
    from concourse import bacc

    fp = mybir.dt.float32
    bf = mybir.dt.bfloat16
    AF = mybir.ActivationFunctionType
    OP = mybir.AluOpType
    AX = mybir.AxisListType

    nc = bacc.Bacc("TRN2")
    xd = nc.dram_tensor("xd", [npairs, 128, XC2], bf, kind="ExternalInput")
    cst = nc.dram_tensor("cst", [128, 704], bf, kind="ExternalInput")
    yd = nc.dram_tensor("yd", [2 * npairs, 128, F], bf, kind="ExternalOutput")

    with tile.TileContext(nc) as tc:
        with (
            tc.tile_pool(name="const", bufs=1) as constp,
            tc.tile_pool(name="xin", bufs=8) as xin,
            tc.tile_pool(name="sm", bufs=6) as sm,
            tc.tile_pool(name="med", bufs=3) as med,
            tc.tile_pool(name="big", bufs=3) as big,
            tc.tile_pool(name="yout", bufs=3) as yout,
            tc.tile_pool(name="ps", bufs=1, space="PSUM") as ps,
        ):
            consts = constp.tile([128, 704], bf)
            nc.sync.dma_start(out=consts, in_=cst[:, :])
            SEGBIG2 = consts[:, 0:256]
            Cm = consts[:, 256:384]
            IDENT = consts[:, 384:512]
            Wp = consts[:, 512:640]
            SEG8 = consts[:, 640:656]
            SEG = consts[:, 656:664]
            ONES32 = consts[:, 664:696]
            EPSC = consts[:, 696:704]

            st = {}

            # Phases are emitted oldest-tile-first each round so every
            # in-order engine drains without same-round cross-engine waits:
            # each op's inputs come from a PREVIOUS round (except the
            # softmax hop chain, which tails the round by design).

            def phase_load(t):          # round t
                xall = xin.tile([128, XC2], bf, tag="x")
                nc.sync.dma_start(out=xall, in_=xd[t])
                st[t] = {"xall": xall}

            def phase_score(t):         # round t+2: pure DVE
                d = st[t]
                xall = d["xall"]
                # 34-segment score sums (32 sj blocks + 2 si rows)
                sv = xall[:, 0:34 * F].rearrange("p (s f) -> p s f", f=F)
                h1 = med.tile([128, 34 * 64], bf, tag="h1")
                nc.vector.tensor_tensor(
                    out=h1.rearrange("p (s f) -> p s f", f=64),
                    in0=sv[:, :, 0:64], in1=sv[:, :, 64:128], op=OP.add)
                h1v = h1.rearrange("p (s f) -> p s f", f=64)
                h2 = med.tile([128, 34 * 32], bf, tag="h2")
                nc.vector.tensor_tensor(
                    out=h2.rearrange("p (s f) -> p s f", f=32),
                    in0=h1v[:, :, 0:32], in1=h1v[:, :, 32:64], op=OP.add)
                h2v = h2.rearrange("p (s f) -> p s f", f=32)
                h3 = med.tile([128, 34 * 16], bf, tag="h3")
                nc.vector.tensor_tensor(
                    out=h3.rearrange("p (s f) -> p s f", f=16),
                    in0=h2v[:, :, 0:16], in1=h2v[:, :, 16:32], op=OP.add)
                s17 = sm.tile([128, 34], fp, tag="s17")
                nc.vector.tensor_reduce(
                    out=s17, in_=h3.rearrange("p (s f) -> p s f", f=16),
                    axis=AX.X, op=OP.add)
                d["s17"] = s17

            def phase_softmax(t):       # round t+4: chain, mostly DVE
                d = st[t]
                xall = d["xall"]
                s17 = d["s17"]
                # scatter si (natural [n,1]) into s-layout via SEG8 + Cm
                Dt2 = sm.tile([128, 32], bf, tag="Dt")
                nc.vector.tensor_scalar(out=Dt2[:, 0:16], in0=SEG8,
                                        scalar1=s17[:, 32:33], scalar2=None,
                                        op0=OP.mult)
                nc.vector.tensor_scalar(out=Dt2[:, 16:32], in0=SEG8,
                                        scalar1=s17[:, 33:34], scalar2=None,
                                        op0=OP.mult)
                si_ps = ps.tile([128, 32], fp, tag="si", bufs=2)
                nc.tensor.matmul(si_ps, lhsT=Cm, rhs=Dt2, start=True, stop=True)
                s2 = sm.tile([128, 32], fp, tag="s2")
                nc.vector.scalar_tensor_tensor(
                    out=s2, in0=s17[:, 0:32], scalar=0.0, in1=si_ps,
                    op0=OP.add, op1=OP.add)
                ls = sm.tile([128, 32], fp, tag="ls")
                nc.vector.scalar_tensor_tensor(
                    out=ls, in0=s2, scalar=ALPHA, in1=s2,
                    op0=OP.mult, op1=OP.max)
                exp_s = sm.tile([128, 32], bf, tag="exp_s")
                nc.scalar.activation(out=exp_s, in_=ls, func=AF.Exp)
                p_s = sm.tile([128, 32], bf, tag="p_s")
                nc.vector.tensor_tensor(out=p_s, in0=exp_s,
                                        in1=xall[:, OFF_ADJ:OFF_ADJ + 32],
                                        op=OP.mult)
                ZT_ps = ps.tile([32, 8], fp, tag="zt", bufs=2)
                nc.tensor.matmul(ZT_ps, lhsT=p_s, rhs=SEG,
                                 start=True, stop=True)
                tz = sm.tile([32, 8], fp, tag="tz")
                nc.scalar.activation(out=tz, in_=ZT_ps, func=AF.Copy,
                                     bias=16.0 * EPS)
                d["p_s"] = p_s
                d["tz"] = tz

            def phase_z(t):             # round t+5: Z scatter
                d = st[t]
                tz = d["tz"]
                zn = sm.tile([128, 2], fp, tag="zn")
                nc.sync.dma_start(out=zn[:, 0:1], in_=tz[0:16, :])
                nc.sync.dma_start(out=zn[:, 1:2], in_=tz[16:32, :])
                d["zn"] = zn

            def phase_xbar(t):          # round t+5
                d = st[t]
                xall = d["xall"]
                zn = d["zn"]
                rz = sm.tile([128, 2], fp, tag="rz")
                nc.vector.reciprocal_approx_fast(rz, zn)
                d["rz"] = rz
                x0z = big.tile([128, 256], bf, tag="x0z")
                for h in (0, 1):
                    nc.vector.tensor_scalar(
                        out=x0z[:, 128 * h:128 * h + 128],
                        in0=xall[:, OFF_X0P + 128 * h:OFF_X0P + 128 * h + 128],
                        scalar1=zn[:, h:h + 1], scalar2=None, op0=OP.mult)
                attsegU = big.tile([128, 256], bf, tag="attsegU")
                p_bc = d["p_s"].rearrange("p (b o) -> p b o", o=1)
                nc.vector.scalar_tensor_tensor(
                    out=attsegU.rearrange("p (b j) -> p b j", j=8),
                    in0=p_bc.to_broadcast([128, 32, 8]), scalar=EPS,
                    in1=SEGBIG2.rearrange("p (b j) -> p b j", j=8),
                    op0=OP.add, op1=OP.mult)
                xbarT_ps = ps.tile([128, 256], fp, tag="xb", bufs=2)
                for h in (0, 1):
                    co = 128 * h
                    nc.tensor.matmul(
                        xbarT_ps[:, co:co + 128],
                        lhsT=x0z[:, co:co + 128], rhs=IDENT,
                        start=True, stop=False, skip_group_check=True)
                    for b in range(BPT):
                        nc.tensor.matmul(
                            xbarT_ps[:, co + 8 * b:co + 8 * b + 8],
                            lhsT=xall[:, 2048 * h + b * F:2048 * h + (b + 1) * F],
                            rhs=attsegU[:, co + 8 * b:co + 8 * b + 8],
                            start=False, stop=(b == BPT - 1),
                            skip_group_check=True)
                d["xbarT_ps"] = xbarT_ps

            def phase_fin(t):           # round t+6: ST + final GEMM
                d = st[t]
                ST = big.tile([128, 256], bf, tag="ST")
                nc.scalar.activation(out=ST, in_=d["xbarT_ps"], func=AF.Copy)
                Y_ps = ps.tile([128, 256], fp, tag="yy", bufs=2)
                for h in (0, 1):
                    co = 128 * h
                    nc.tensor.matmul(Y_ps[:, co:co + 128],
                                     lhsT=ST[:, co:co + 128], rhs=Wp,
                                     start=True, stop=True)
                d["Y_ps"] = Y_ps

            def phase_act(t):           # round t+7: elu activations
                d = st[t]
                rz = d["rz"]
                Y_ps = d["Y_ps"]
                e = yout.tile([128, 256], bf, tag="e")
                r = yout.tile([128, 256], bf, tag="r")
                for h in (0, 1):
                    co = 128 * h
                    nc.scalar.activation(out=e[:, co:co + 128],
                                         in_=Y_ps[:, co:co + 128],
                                         func=AF.Exp, scale=rz[:, h:h + 1])
                    nc.scalar.activation(out=r[:, co:co + 128],
                                         in_=Y_ps[:, co:co + 128],
                                         func=AF.Relu, scale=rz[:, h:h + 1])
                v = yout.tile([128, 256], bf, tag="v")
                nc.scalar.activation(out=v, in_=e, func=AF.Relu,
                                     scale=-1.0, bias=1.0)
                d["r"] = r
                d["v"] = v

            def phase_out(t):           # round t+8: y + store
                d = st[t]
                y = yout.tile([128, 256], bf, tag="y")
                nc.vector.tensor_tensor(out=y, in0=d["r"], in1=d["v"],
                                        op=OP.subtract)
                nc.sync.dma_start(out=yd[2 * t], in_=y[:, 0:128])
                nc.sync.dma_start(out=yd[2 * t + 1], in_=y[:, 128:256])
                del st[t]

            for r in range(npairs + 9):
                if 0 <= r - 9 < npairs:
                    phase_out(r - 9)
                if 0 <= r - 8 < npairs:
                    phase_act(r - 8)
                if 0 <= r - 7 < npairs:
                    phase_fin(r - 7)
                if 0 <= r - 6 < npairs:
                    phase_xbar(r - 6)
                if 0 <= r - 5 < npairs:
                    phase_z(r - 5)
                if 0 <= r - 2 < npairs:
                    phase_score(r - 2)
                if 0 <= r - 4 < npairs:
                    phase_softmax(r - 4)
                if r < npairs:
                    phase_load(r)

    if finalize:
        nc.finalize()
    return nc


def _get_nc(npairs=NPAIRS):
    if npairs not in _NC_CACHE:
        _NC_CACHE[npairs] = _build_nc(npairs)
    return _NC_CACHE[npairs]


def _shard_inputs(orignal_x, x, adj, W, a, ncores=NCORES, ntiles=NTILES):
    f32 = np.float32
    rpc = TILE * ntiles          # padded rows per core (6400)
    x = np.asarray(x, f32)
    x0 = np.asarray(orignal_x, f32)
    adj = np.asarray(adj, np.int32)
    cst, wa1, wa2 = _consts_full_np(W, a)
    wa1_f = wa1.astype(f32)
    wa2_f = wa2.astype(f32)
    n = x.shape[0]
    assert n <= RPC_REAL * ncores

    in_maps = []
    for c in range(ncores):
        lo = c * RPC_REAL
        hi = min((c + 1) * RPC_REAL, n)
        rows = hi - lo
        xc = x[lo:hi]
        x0c = x0[lo:hi]
        adjc = adj[lo:hi]
        if rows < rpc:
            pad = rpc - rows
            xc = np.concatenate([xc, np.zeros((pad, K, F), f32)])
            x0c = np.concatenate([x0c, np.zeros((pad, F), f32)])
            adjc = np.concatenate([adjc, np.zeros((pad, K), np.int32)])
        # per-tile packs [50, 128, *]
        xp = (xc * wa2_f[None, None, :]).astype(BF16)
        xp50 = xp.reshape(ntiles, 16, 8, K, F).transpose(
            0, 2, 3, 1, 4).reshape(ntiles, 128, BPT * F)
        x0w1 = (x0c * wa1_f[None, :]).astype(BF16).reshape(ntiles, 128, F)
        x0p = (x0c * wa2_f[None, :]).astype(BF16).reshape(ntiles, 128, F)
        adj50 = adjc.astype(BF16).reshape(ntiles, 16, 8, K).transpose(
            0, 2, 3, 1).reshape(ntiles, 128, K)
        xdev = np.empty((NPAIRS, 128, XC2), BF16)
        xdev[:, :, 0:2048] = xp50[0::2]
        xdev[:, :, 2048:4096] = xp50[1::2]
        xdev[:, :, OFF_X0W1:OFF_X0W1 + F] = x0w1[0::2]
        xdev[:, :, OFF_X0W1 + F:OFF_X0W1 + 2 * F] = x0w1[1::2]
        xdev[:, :, OFF_X0P:OFF_X0P + F] = x0p[0::2]
        xdev[:, :, OFF_X0P + F:OFF_X0P + 2 * F] = x0p[1::2]
        xdev[:, :, OFF_ADJ:OFF_ADJ + K] = adj50[0::2]
        xdev[:, :, OFF_ADJ + K:OFF_ADJ + 2 * K] = adj50[1::2]
        in_maps.append({"xd": xdev, "cst": cst})
    return in_maps


_LAST_RESULTS = None


def kernel(orignal_x, x, adj, W, a):
    import os
    os.environ.setdefault("JAX_PLATFORMS", "")
    from concourse.bass_utils import run_bass_kernel_spmd

    global _LAST_RESULTS
    nc = _get_nc()
    in_maps = _shard_inputs(orignal_x, x, adj, W, a)
    res = run_bass_kernel_spmd(nc, in_maps, list(range(NCORES)))
    _LAST_RESULTS = res
    y = np.concatenate(
        [np.asarray(r["yd"]).astype(np.float32).reshape(
            TILE * NTILES, F)[:RPC_REAL]
         for r in res.results], axis=0)
    return np.ascontiguousarray(y[:N])


# revision 11
# speedup vs baseline: 1.1498x; 1.1165x over previous
"""GAT attention kernel for 8 trn2 NeuronCores (Bass/Tile), bf16 q-layout,
paired-tile pipeline.

Math (restructured from the reference):
    wa1 = W @ a1, wa2 = W @ a2                      (host, weight folding)
    x'  = x * wa2[f]   (host)                        so  sj[n,k] = sum_f x'[n,k,f]
    x0w1 = x0 * wa1[f] (host)                        so  si[n]   = sum_f x0w1[n,f]
    x0' = x0 * wa2[f]  (host),  W' = W / wa2[:,None] (host)
    s       = si + sj
    p       = exp(prelu(s, 0.2)) * adj
    Z'      = sum_k p + 16*EPS                       (per node)
    U       = sum_k (p_k+EPS) * x'_k  +  Z' * x0'    (unnormalized, PSUM)
    out     = elu((U @ W') / Z')                     (/Z' via ACT per-partition scale)
    elu(z)  = relu(z) - relu(1 - exp(z))

Sharding: node dim N padded 50000 -> 51200 = 8 cores * 25 pairs * 256 rows.
Each round processes a PAIR of 128-row tiles from one 9280B/partition DMA;
small ops (score TT/reduce chain, softmax, activations) run once per pair,
halving fixed per-instruction overheads.

Per 128-row tile the 2048 (n,k) pairs form 16 blocks of [128 q, 128 f] bf16
(q = 16*(n%8) + k, block b = n_tile//8). Scores for the whole pair (32 sj
segments + 2 si segments) come from 2 halving TTs + ONE segmented
tensor_reduce on DVE.
"""

import numpy as np
import ml_dtypes

N, K, F = 50000, 16, 128
ALPHA = 0.2
NCORES = 8
TILE = 128
NTILES = 50                  # padded (6400 rows/core, 6272 real)
NPAIRS = NTILES // 2
RPC_REAL = 6272              # real rows per core
BPT = K                      # nk-blocks per tile = 16
XCOLS = BPT * F + F + F + K  # per-tile bf16 cols = 2320
XC2 = 2 * XCOLS              # 4640
# pair-tile column map
OFF_XB = 2 * BPT * F         # 4096: end of the two x' block regions
OFF_X0W1 = 4096              # x0w1 A | x0w1 B  (score segments 32, 33)
OFF_X0P = 4352               # x0' A | x0' B
OFF_ADJ = 4608               # adj A | adj B (s-layout)
EPS = 1e-12

BF16 = ml_dtypes.bfloat16

_NC_CACHE = {}


def _consts_np():
    p = np.arange(128)
    j8 = np.arange(8)
    b16 = np.arange(16)
    ident = np.eye(128, dtype=np.float32)
    Cm = (p[:, None] % 8 == p[None, :] // 16).astype(np.float32)
    segbig = (p[:, None] // 16 == (p[None, :] % 8)).astype(np.float32)
    seg = (p[:, None] // 16 == j8[None, :]).astype(np.float32)
    seg8 = (p[:, None] // 8 == b16[None, :]).astype(np.float32)
    return ident, Cm, segbig, seg, seg8


def _consts_full_np(W, a):
    W = np.asarray(W, np.float64)
    a = np.asarray(a, np.float64)
    wa1 = W @ a[:F, 0]
    wa2 = W @ a[F:, 0]
    Wp = W / wa2[:, None]
    ident, Cm, segbig, seg, seg8 = _consts_np()
    ones32 = np.ones((128, 32), dtype=np.float32)
    epsc = np.full((128, 8), 16.0 * EPS / 128.0, dtype=np.float32)
    cst = np.concatenate(
        [segbig, segbig, Cm, ident, Wp.astype(np.float32), seg8, seg,
         ones32, epsc], axis=1)
    return np.ascontiguousarray(cst).astype(BF16), wa1, wa2  # [128, 704]


def _build_nc(npairs=NPAIRS, finalize=True):
    import concourse.mybir as mybir
    import concourse.tile as tile
    from concourse import bacc

    fp = mybir.dt.float32
    bf = mybir.dt.bfloat16
    AF = mybir.ActivationFunctionType
    OP = mybir.AluOpType
    AX = mybir.AxisListType

    nc = bacc.Bacc("TRN2")
    xd = nc.dram_tensor("xd", [npairs, 128, XC2], bf, kind="ExternalInput")
    cst = nc.dram_tensor("cst", [128, 704], bf, kind="ExternalInput")
    yd = nc.dram_tensor("yd", [2 * npairs, 128, F], bf, kind="ExternalOutput")

    with tile.TileContext(nc) as tc:
        with (
            tc.tile_pool(name="const", bufs=1) as constp,
            tc.tile_pool(name="xin", bufs=8) as xin,
            tc.tile_pool(name="sm", bufs=6) as sm,
            tc.tile_pool(name="med", bufs=3) as med,
            tc.tile_pool(name="big", bufs=3) as big,
            tc.tile_pool(name="yout", bufs=3) as yout,
            tc.tile_pool(name="ps", bufs=1, space="PSUM") as ps,
        ):
            consts = constp.tile([128, 704], bf)
            nc.sync.dma_start(out=consts, in_=cst[:, :])
            SEGBIG2 = consts[:, 0:256]
            Cm = consts[:, 256:384]
            IDENT = consts[:, 384:512]
            Wp = consts[:, 512:640]
            SEG8 = consts[:, 640:656]
            SEG = consts[:, 656:664]
            ONES32 = consts[:, 664:696]
            EPSC = consts[:, 696:704]

            st = {}

            # Phases are emitted oldest-tile-first each round so every
            # in-order engine drains without same-round cross-engine waits:
            # each op's inputs come from a PREVIOUS round (except the
            # softmax hop chain, which tails the round by design).

            def phase_load(t):          # round t
                xall = xin.tile([128, XC2], bf, tag="x")
                nc.sync.dma_start(out=xall, in_=xd[t])
                st[t] = {"xall": xall}

            def phase_score(t):         # round t+2: pure DVE
                d = st[t]
                xall = d["xall"]
                # 34-segment score sums (32 sj blocks + 2 si rows)
                sv = xall[:, 0:34 * F].rearrange("p (s f) -> p s f", f=F)
                h1 = med.tile([128, 34 * 64], bf, tag="h1")
                nc.vector.tensor_tensor(
                    out=h1.rearrange("p (s f) -> p s f", f=64),
                    in0=sv[:, :, 0:64], in1=sv[:, :, 64:128], op=OP.add)
                h1v = h1.rearrange("p (s f) -> p s f", f=64)
                h2 = med.tile([128, 34 * 32], bf, tag="h2")
                nc.vector.tensor_tensor(
                    out=h2.rearrange("p (s f) -> p s f", f=32),
                    in0=h1v[:, :, 0:32], in1=h1v[:, :, 32:64], op=OP.add)
                h2v = h2.rearrange("p (s f) -> p s f", f=32)
                h3 = med.tile([128, 34 * 16], bf, tag="h3")
                nc.vector.tensor_tensor(
                    out=h3.rearrange("p (s f) -> p s f", f=16),
                    in0=h2v[:, :, 0:16], in1=h2v[:, :, 16:32], op=OP.add)
                s17 = sm.tile([128, 34], fp, tag="s17")
                nc.vector.tensor_reduce(
                    out=s17, in_=h3.rearrange("p (s f) -> p s f", f=16),
                    axis=AX.X, op=OP.add)
                d["s17"] = s17

            def phase_softmax(t):       # round t+4: chain, mostly DVE
                d = st[t]
                xall = d["xall"]
                s17 = d["s17"]
                # scatter si (natural [n,1]) into s-layout via SEG8 + Cm
                Dt2 = sm.tile([128, 32], bf, tag="Dt")
                nc.vector.tensor_scalar(out=Dt2[:, 0:16], in0=SEG8,
                                        scalar1=s17[:, 32:33], scalar2=None,
                                        op0=OP.mult)
                nc.vector.tensor_scalar(out=Dt2[:, 16:32], in0=SEG8,
                                        scalar1=s17[:, 33:34], scalar2=None,
                                        op0=OP.mult)
                si_ps = ps.tile([128, 32], fp, tag="si", bufs=2)
                nc.tensor.matmul(si_ps, lhsT=Cm, rhs=Dt2,
                                 start=True, stop=False, skip_group_check=True)
                nc.tensor.matmul(si_ps, lhsT=IDENT,
                                 rhs=xall[:, OFF_ADJ:OFF_ADJ + 32],
                                 start=False, stop=True, skip_group_check=True)
                s2 = sm.tile([128, 32], fp, tag="s2")
                nc.vector.scalar_tensor_tensor(
                    out=s2, in0=s17[:, 0:32], scalar=0.0, in1=si_ps,
                    op0=OP.add, op1=OP.add)
                ls = sm.tile([128, 32], fp, tag="ls")
                nc.vector.scalar_tensor_tensor(
                    out=ls, in0=s2, scalar=ALPHA, in1=s2,
                    op0=OP.mult, op1=OP.max)
                p_s = sm.tile([128, 32], bf, tag="p_s")
                nc.scalar.activation(out=p_s, in_=ls, func=AF.Exp)
                ZT_ps = ps.tile([32, 8], fp, tag="zt", bufs=2)
                nc.tensor.matmul(ZT_ps, lhsT=p_s, rhs=SEG,
                                 start=True, stop=True)
                tz = sm.tile([32, 8], fp, tag="tz")
                nc.scalar.activation(out=tz, in_=ZT_ps, func=AF.Copy,
                                     bias=16.0 * EPS)
                d["p_s"] = p_s
                d["tz"] = tz

            def phase_z(t):             # round t+5: Z scatter
                d = st[t]
                tz = d["tz"]
                zn = sm.tile([128, 2], fp, tag="zn")
                nc.sync.dma_start(out=zn[:, 0:1], in_=tz[0:16, :])
                nc.sync.dma_start(out=zn[:, 1:2], in_=tz[16:32, :])
                d["zn"] = zn

            def phase_xbar(t):          # round t+5
                d = st[t]
                xall = d["xall"]
                zn = d["zn"]
                rz = sm.tile([128, 2], fp, tag="rz")
                nc.vector.reciprocal_approx_fast(rz, zn)
                d["rz"] = rz
                x0z = big.tile([128, 256], bf, tag="x0z")
                for h in (0, 1):
                    nc.vector.tensor_scalar(
                        out=x0z[:, 128 * h:128 * h + 128],
                        in0=xall[:, OFF_X0P + 128 * h:OFF_X0P + 128 * h + 128],
                        scalar1=zn[:, h:h + 1], scalar2=None, op0=OP.mult)
                attsegU = big.tile([128, 256], bf, tag="attsegU")
                p_bc = d["p_s"].rearrange("p (b o) -> p b o", o=1)
                nc.vector.scalar_tensor_tensor(
                    out=attsegU.rearrange("p (b j) -> p b j", j=8),
                    in0=p_bc.to_broadcast([128, 32, 8]), scalar=EPS,
                    in1=SEGBIG2.rearrange("p (b j) -> p b j", j=8),
                    op0=OP.add, op1=OP.mult)
                xbarT_ps = ps.tile([128, 256], fp, tag="xb", bufs=2)
                for h in (0, 1):
                    co = 128 * h
                    nc.tensor.matmul(
                        xbarT_ps[:, co:co + 128],
                        lhsT=x0z[:, co:co + 128], rhs=IDENT,
                        start=True, stop=False, skip_group_check=True)
                    for b in range(BPT):
                        nc.tensor.matmul(
                            xbarT_ps[:, co + 8 * b:co + 8 * b + 8],
                            lhsT=xall[:, 2048 * h + b * F:2048 * h + (b + 1) * F],
                            rhs=attsegU[:, co + 8 * b:co + 8 * b + 8],
                            start=False, stop=(b == BPT - 1),
                            skip_group_check=True)
                d["xbarT_ps"] = xbarT_ps

            def phase_fin(t):           # round t+6: ST + final GEMM
                d = st[t]
                ST = big.tile([128, 256], bf, tag="ST")
                nc.scalar.activation(out=ST, in_=d["xbarT_ps"], func=AF.Copy)
                Y_ps = ps.tile([128, 256], fp, tag="yy", bufs=2)
                for h in (0, 1):
                    co = 128 * h
                    nc.tensor.matmul(Y_ps[:, co:co + 128],
                                     lhsT=ST[:, co:co + 128], rhs=Wp,
                                     start=True, stop=True)
                d["Y_ps"] = Y_ps

            def phase_act(t):           # round t+7: elu activations
                d = st[t]
                rz = d["rz"]
                Y_ps = d["Y_ps"]
                e = yout.tile([128, 256], bf, tag="e")
                r = yout.tile([128, 256], bf, tag="r")
                for h in (0, 1):
                    co = 128 * h
                    nc.scalar.activation(out=e[:, co:co + 128],
                                         in_=Y_ps[:, co:co + 128],
                                         func=AF.Exp, scale=rz[:, h:h + 1])
                    nc.scalar.activation(out=r[:, co:co + 128],
                                         in_=Y_ps[:, co:co + 128],
                                         func=AF.Relu, scale=rz[:, h:h + 1])
                v = yout.tile([128, 256], bf, tag="v")
                nc.scalar.activation(out=v, in_=e, func=AF.Relu,
                                     scale=-1.0, bias=1.0)
                d["r"] = r
                d["v"] = v

            def phase_out(t):           # round t+8: y + store
                d = st[t]
                y = yout.tile([128, 256], bf, tag="y")
                nc.vector.tensor_tensor(out=y, in0=d["r"], in1=d["v"],
                                        op=OP.subtract)
                nc.sync.dma_start(out=yd[2 * t], in_=y[:, 0:128])
                nc.sync.dma_start(out=yd[2 * t + 1], in_=y[:, 128:256])
                del st[t]

            for r in range(npairs + 9):
                if 0 <= r - 9 < npairs:
                    phase_out(r - 9)
                if 0 <= r - 8 < npairs:
                    phase_act(r - 8)
                if 0 <= r - 7 < npairs:
                    phase_fin(r - 7)
                if 0 <= r - 6 < npairs:
                    phase_xbar(r - 6)
                if 0 <= r - 5 < npairs:
                    phase_z(r - 5)
                if 0 <= r - 2 < npairs:
                    phase_score(r - 2)
                if 0 <= r - 4 < npairs:
                    phase_softmax(r - 4)
                if r < npairs:
                    phase_load(r)

    if finalize:
        nc.finalize()
    return nc


def _get_nc(npairs=NPAIRS):
    if npairs not in _NC_CACHE:
        _NC_CACHE[npairs] = _build_nc(npairs)
    return _NC_CACHE[npairs]


def _shard_inputs(orignal_x, x, adj, W, a, ncores=NCORES, ntiles=NTILES):
    f32 = np.float32
    rpc = TILE * ntiles          # padded rows per core (6400)
    x = np.asarray(x, f32)
    x0 = np.asarray(orignal_x, f32)
    adj = np.asarray(adj, np.int32)
    cst, wa1, wa2 = _consts_full_np(W, a)
    wa1_f = wa1.astype(f32)
    wa2_f = wa2.astype(f32)
    n = x.shape[0]
    assert n <= RPC_REAL * ncores

    in_maps = []
    for c in range(ncores):
        lo = c * RPC_REAL
        hi = min((c + 1) * RPC_REAL, n)
        rows = hi - lo
        xc = x[lo:hi]
        x0c = x0[lo:hi]
        adjc = adj[lo:hi]
        if rows < rpc:
            pad = rpc - rows
            xc = np.concatenate([xc, np.zeros((pad, K, F), f32)])
            x0c = np.concatenate([x0c, np.zeros((pad, F), f32)])
            adjc = np.concatenate([adjc, np.zeros((pad, K), np.int32)])
        # per-tile packs [50, 128, *]
        xp = (xc * wa2_f[None, None, :]).astype(BF16)
        xp50 = xp.reshape(ntiles, 16, 8, K, F).transpose(
            0, 2, 3, 1, 4).reshape(ntiles, 128, BPT * F)
        x0w1 = (x0c * wa1_f[None, :]).astype(BF16).reshape(ntiles, 128, F)
        x0p = (x0c * wa2_f[None, :]).astype(BF16).reshape(ntiles, 128, F)
        adj50 = ((adjc - 1.0) * 1000.0).astype(BF16).reshape(
            ntiles, 16, 8, K).transpose(0, 2, 3, 1).reshape(ntiles, 128, K)
        xdev = np.empty((NPAIRS, 128, XC2), BF16)
        xdev[:, :, 0:2048] = xp50[0::2]
        xdev[:, :, 2048:4096] = xp50[1::2]
        xdev[:, :, OFF_X0W1:OFF_X0W1 + F] = x0w1[0::2]
        xdev[:, :, OFF_X0W1 + F:OFF_X0W1 + 2 * F] = x0w1[1::2]
        xdev[:, :, OFF_X0P:OFF_X0P + F] = x0p[0::2]
        xdev[:, :, OFF_X0P + F:OFF_X0P + 2 * F] = x0p[1::2]
        xdev[:, :, OFF_ADJ:OFF_ADJ + K] = adj50[0::2]
        xdev[:, :, OFF_ADJ + K:OFF_ADJ + 2 * K] = adj50[1::2]
        in_maps.append({"xd": xdev, "cst": cst})
    return in_maps


_LAST_RESULTS = None


def kernel(orignal_x, x, adj, W, a):
    import os
    os.environ.setdefault("JAX_PLATFORMS", "")
    from concourse.bass_utils import run_bass_kernel_spmd

    global _LAST_RESULTS
    nc = _get_nc()
    in_maps = _shard_inputs(orignal_x, x, adj, W, a)
    res = run_bass_kernel_spmd(nc, in_maps, list(range(NCORES)))
    _LAST_RESULTS = res
    y = np.concatenate(
        [np.asarray(r["yd"]).astype(np.float32).reshape(
            TILE * NTILES, F)[:RPC_REAL]
         for r in res.results], axis=0)
    return np.ascontiguousarray(y[:N])


# revision 12
# speedup vs baseline: 1.2482x; 1.0856x over previous
"""GAT attention kernel for 8 trn2 NeuronCores (Bass/Tile), bf16 q-layout,
paired-tile pipeline.

Math (restructured from the reference):
    wa1 = W @ a1, wa2 = W @ a2                      (host, weight folding)
    x'  = x * wa2[f]   (host)                        so  sj[n,k] = sum_f x'[n,k,f]
    x0w1 = x0 * wa1[f] (host)                        so  si[n]   = sum_f x0w1[n,f]
    x0' = x0 * wa2[f]  (host),  W' = W / wa2[:,None] (host)
    s       = si + sj
    p       = exp(prelu(s, 0.2)) * adj
    Z'      = sum_k p + 16*EPS                       (per node)
    U       = sum_k (p_k+EPS) * x'_k  +  Z' * x0'    (unnormalized, PSUM)
    out     = elu((U @ W') / Z')                     (/Z' via ACT per-partition scale)
    elu(z)  = relu(z) - relu(1 - exp(z))

Sharding: node dim N padded 50000 -> 51200 = 8 cores * 25 pairs * 256 rows.
Each round processes a PAIR of 128-row tiles from one 9280B/partition DMA;
small ops (score TT/reduce chain, softmax, activations) run once per pair,
halving fixed per-instruction overheads.

Per 128-row tile the 2048 (n,k) pairs form 16 blocks of [128 q, 128 f] bf16
(q = 16*(n%8) + k, block b = n_tile//8). Scores for the whole pair (32 sj
segments + 2 si segments) come from 2 halving TTs + ONE segmented
tensor_reduce on DVE.
"""

import numpy as np
import ml_dtypes

N, K, F = 50000, 16, 128
ALPHA = 0.2
NCORES = 8
TILE = 128
NTILES = 50                  # padded (6400 rows/core, 6272 real)
NPAIRS = NTILES // 2
RPC_REAL = 6272              # real rows per core
BPT = K                      # nk-blocks per tile = 16
XCOLS = BPT * F + F + F + K  # per-tile bf16 cols = 2320
XC2 = 2 * XCOLS              # 4640
# pair-tile column map
OFF_XB = 2 * BPT * F         # 4096: end of the two x' block regions
OFF_X0W1 = 4096              # x0w1 A | x0w1 B  (score segments 32, 33)
OFF_X0P = 4352               # x0' A | x0' B
OFF_ADJ = 4608               # adj A | adj B (s-layout)
EPS = 1e-12

BF16 = ml_dtypes.bfloat16

_NC_CACHE = {}


def _consts_np():
    p = np.arange(128)
    j8 = np.arange(8)
    b16 = np.arange(16)
    ident = np.eye(128, dtype=np.float32)
    Cm = (p[:, None] % 8 == p[None, :] // 16).astype(np.float32)
    segbig = (p[:, None] // 16 == (p[None, :] % 8)).astype(np.float32)
    seg = (p[:, None] // 16 == j8[None, :]).astype(np.float32)
    seg8 = (p[:, None] // 8 == b16[None, :]).astype(np.float32)
    return ident, Cm, segbig, seg, seg8


def _consts_full_np(W, a):
    W = np.asarray(W, np.float64)
    a = np.asarray(a, np.float64)
    wa1 = W @ a[:F, 0]
    wa2 = W @ a[F:, 0]
    Wp = W / wa2[:, None]
    ident, Cm, segbig, seg, seg8 = _consts_np()
    ones32 = np.ones((128, 32), dtype=np.float32)
    epsc = np.full((128, 8), 16.0 * EPS / 128.0, dtype=np.float32)
    cst = np.concatenate(
        [segbig, segbig, Cm, ident, Wp.astype(np.float32), seg8, seg,
         ones32, epsc], axis=1)
    return np.ascontiguousarray(cst).astype(BF16), wa1, wa2  # [128, 704]


def _build_nc(npairs=NPAIRS, finalize=True):
    import concourse.mybir as mybir
    import concourse.tile as tile
    from concourse import bacc

    fp = mybir.dt.float32
    bf = mybir.dt.bfloat16
    AF = mybir.ActivationFunctionType
    OP = mybir.AluOpType
    AX = mybir.AxisListType

    nc = bacc.Bacc("TRN2")
    xd = nc.dram_tensor("xd", [npairs, 128, XC2], bf, kind="ExternalInput")
    cst = nc.dram_tensor("cst", [128, 704], bf, kind="ExternalInput")
    yd = nc.dram_tensor("yd", [2 * npairs, 128, F], bf, kind="ExternalOutput")

    with tile.TileContext(nc) as tc:
        with (
            tc.tile_pool(name="const", bufs=1) as constp,
            tc.tile_pool(name="xin", bufs=9) as xin,
            tc.tile_pool(name="sm", bufs=6) as sm,
            tc.tile_pool(name="med", bufs=3) as med,
            tc.tile_pool(name="big", bufs=3) as big,
            tc.tile_pool(name="yout", bufs=3) as yout,
            tc.tile_pool(name="ps", bufs=1, space="PSUM") as ps,
        ):
            consts = constp.tile([128, 704], bf)
            nc.sync.dma_start(out=consts, in_=cst[:, :])
            SEGBIG2 = consts[:, 0:256]
            Cm = consts[:, 256:384]
            IDENT = consts[:, 384:512]
            Wp = consts[:, 512:640]
            SEG8 = consts[:, 640:656]
            SEG = consts[:, 656:664]
            ONES32 = consts[:, 664:696]
            EPSC = consts[:, 696:704]

            st = {}

            # Phases are emitted oldest-tile-first each round so every
            # in-order engine drains without same-round cross-engine waits:
            # each op's inputs come from a PREVIOUS round (except the
            # softmax hop chain, which tails the round by design).

            def phase_load(t):          # round t
                xall = xin.tile([128, XC2], bf, tag="x")
                nc.sync.dma_start(out=xall, in_=xd[t])
                st[t] = {"xall": xall}

            def phase_score(t):         # round t+2: pure DVE
                d = st[t]
                xall = d["xall"]
                # 34-segment score sums (32 sj blocks + 2 si rows)
                sv = xall[:, 0:34 * F].rearrange("p (s f) -> p s f", f=F)
                h1 = med.tile([128, 34 * 64], bf, tag="h1")
                nc.vector.tensor_tensor(
                    out=h1.rearrange("p (s f) -> p s f", f=64),
                    in0=sv[:, :, 0:64], in1=sv[:, :, 64:128], op=OP.add)
                h1v = h1.rearrange("p (s f) -> p s f", f=64)
                h2 = med.tile([128, 34 * 32], bf, tag="h2")
                nc.vector.tensor_tensor(
                    out=h2.rearrange("p (s f) -> p s f", f=32),
                    in0=h1v[:, :, 0:32], in1=h1v[:, :, 32:64], op=OP.add)
                h2v = h2.rearrange("p (s f) -> p s f", f=32)
                h3 = med.tile([128, 34 * 16], bf, tag="h3")
                nc.vector.tensor_tensor(
                    out=h3.rearrange("p (s f) -> p s f", f=16),
                    in0=h2v[:, :, 0:16], in1=h2v[:, :, 16:32], op=OP.add)
                s17 = sm.tile([128, 34], fp, tag="s17")
                nc.vector.tensor_reduce(
                    out=s17, in_=h3.rearrange("p (s f) -> p s f", f=16),
                    axis=AX.X, op=OP.add)
                d["s17"] = s17

            def phase_softmax(t):       # round t+4: chain, mostly DVE
                d = st[t]
                xall = d["xall"]
                s17 = d["s17"]
                # scatter si (natural [n,1]) into s-layout via SEG8 + Cm
                Dt2 = sm.tile([128, 32], bf, tag="Dt")
                nc.vector.tensor_scalar(out=Dt2[:, 0:16], in0=SEG8,
                                        scalar1=s17[:, 32:33], scalar2=None,
                                        op0=OP.mult)
                nc.vector.tensor_scalar(out=Dt2[:, 16:32], in0=SEG8,
                                        scalar1=s17[:, 33:34], scalar2=None,
                                        op0=OP.mult)
                si_ps = ps.tile([128, 32], fp, tag="si", bufs=2)
                nc.tensor.matmul(si_ps, lhsT=Cm, rhs=Dt2,
                                 start=True, stop=False, skip_group_check=True)
                nc.tensor.matmul(si_ps, lhsT=IDENT,
                                 rhs=xall[:, OFF_ADJ:OFF_ADJ + 32],
                                 start=False, stop=True, skip_group_check=True)
                s2 = sm.tile([128, 32], fp, tag="s2")
                nc.vector.scalar_tensor_tensor(
                    out=s2, in0=s17[:, 0:32], scalar=0.0, in1=si_ps,
                    op0=OP.add, op1=OP.add)
                ls = sm.tile([128, 32], fp, tag="ls")
                nc.vector.scalar_tensor_tensor(
                    out=ls, in0=s2, scalar=ALPHA, in1=s2,
                    op0=OP.mult, op1=OP.max)
                p_s = sm.tile([128, 32], bf, tag="p_s")
                nc.scalar.activation(out=p_s, in_=ls, func=AF.Exp)
                ZT_ps = ps.tile([32, 8], fp, tag="zt", bufs=2)
                nc.tensor.matmul(ZT_ps, lhsT=p_s, rhs=SEG,
                                 start=True, stop=True)
                tz = sm.tile([32, 8], fp, tag="tz")
                nc.scalar.activation(out=tz, in_=ZT_ps, func=AF.Copy,
                                     bias=16.0 * EPS)
                d["p_s"] = p_s
                d["tz"] = tz

            def phase_z(t):             # round t+5: Z scatter
                d = st[t]
                tz = d["tz"]
                zn = sm.tile([128, 2], fp, tag="zn")
                nc.gpsimd.dma_start(out=zn[:, 0:1], in_=tz[0:16, :])
                nc.gpsimd.dma_start(out=zn[:, 1:2], in_=tz[16:32, :])
                d["zn"] = zn

            def phase_xbar(t):          # round t+5
                d = st[t]
                xall = d["xall"]
                zn = d["zn"]
                rz = sm.tile([128, 2], fp, tag="rz")
                nc.vector.reciprocal_approx_fast(rz, zn)
                d["rz"] = rz
                x0z = big.tile([128, 256], bf, tag="x0z")
                for h in (0, 1):
                    nc.vector.tensor_scalar(
                        out=x0z[:, 128 * h:128 * h + 128],
                        in0=xall[:, OFF_X0P + 128 * h:OFF_X0P + 128 * h + 128],
                        scalar1=zn[:, h:h + 1], scalar2=None, op0=OP.mult)
                attsegU = big.tile([128, 256], bf, tag="attsegU")
                p_bc = d["p_s"].rearrange("p (b o) -> p b o", o=1)
                nc.vector.scalar_tensor_tensor(
                    out=attsegU.rearrange("p (b j) -> p b j", j=8),
                    in0=p_bc.to_broadcast([128, 32, 8]), scalar=EPS,
                    in1=SEGBIG2.rearrange("p (b j) -> p b j", j=8),
                    op0=OP.add, op1=OP.mult)
                xbarT_ps = ps.tile([128, 256], fp, tag="xb", bufs=2)
                for h in (0, 1):
                    co = 128 * h
                    nc.tensor.matmul(
                        xbarT_ps[:, co:co + 128],
                        lhsT=x0z[:, co:co + 128], rhs=IDENT,
                        start=True, stop=False, skip_group_check=True)
                    for b in range(BPT):
                        nc.tensor.matmul(
                            xbarT_ps[:, co + 8 * b:co + 8 * b + 8],
                            lhsT=xall[:, 2048 * h + b * F:2048 * h + (b + 1) * F],
                            rhs=attsegU[:, co + 8 * b:co + 8 * b + 8],
                            start=False, stop=(b == BPT - 1),
                            skip_group_check=True)
                d["xbarT_ps"] = xbarT_ps

            def phase_fin(t):           # round t+6: ST + final GEMM
                d = st[t]
                ST = big.tile([128, 256], bf, tag="ST")
                nc.scalar.activation(out=ST, in_=d["xbarT_ps"], func=AF.Copy)
                Y_ps = ps.tile([128, 256], fp, tag="yy", bufs=2)
                for h in (0, 1):
                    co = 128 * h
                    nc.tensor.matmul(Y_ps[:, co:co + 128],
                                     lhsT=ST[:, co:co + 128], rhs=Wp,
                                     start=True, stop=True)
                d["Y_ps"] = Y_ps

            def phase_act(t):           # round t+7: elu activations
                d = st[t]
                rz = d["rz"]
                Y_ps = d["Y_ps"]
                e = yout.tile([128, 256], bf, tag="e")
                r = yout.tile([128, 256], bf, tag="r")
                for h in (0, 1):
                    co = 128 * h
                    nc.scalar.activation(out=e[:, co:co + 128],
                                         in_=Y_ps[:, co:co + 128],
                                         func=AF.Exp, scale=rz[:, h:h + 1])
                    nc.scalar.activation(out=r[:, co:co + 128],
                                         in_=Y_ps[:, co:co + 128],
                                         func=AF.Relu, scale=rz[:, h:h + 1])
                v = yout.tile([128, 256], bf, tag="v")
                nc.scalar.activation(out=v, in_=e, func=AF.Relu,
                                     scale=-1.0, bias=1.0)
                d["r"] = r
                d["v"] = v

            def phase_out(t):           # round t+8: y + store
                d = st[t]
                y = yout.tile([128, 256], bf, tag="y")
                nc.vector.tensor_tensor(out=y, in0=d["r"], in1=d["v"],
                                        op=OP.subtract)
                nc.gpsimd.dma_start(out=yd[2 * t], in_=y[:, 0:128])
                nc.gpsimd.dma_start(out=yd[2 * t + 1], in_=y[:, 128:256])
                del st[t]

            for r in range(npairs + 10):
                if 0 <= r - 4 < npairs:
                    phase_softmax(r - 4)
                if 0 <= r - 5 < npairs:
                    phase_z(r - 5)
                if 0 <= r - 10 < npairs:
                    phase_out(r - 10)
                if 0 <= r - 9 < npairs:
                    phase_act(r - 9)
                if 0 <= r - 8 < npairs:
                    phase_fin(r - 8)
                if 0 <= r - 7 < npairs:
                    phase_xbar(r - 7)
                if 0 <= r - 2 < npairs:
                    phase_score(r - 2)
                if r < npairs:
                    phase_load(r)

    if finalize:
        nc.finalize()
    return nc


def _get_nc(npairs=NPAIRS):
    if npairs not in _NC_CACHE:
        _NC_CACHE[npairs] = _build_nc(npairs)
    return _NC_CACHE[npairs]


def _shard_inputs(orignal_x, x, adj, W, a, ncores=NCORES, ntiles=NTILES):
    f32 = np.float32
    rpc = TILE * ntiles          # padded rows per core (6400)
    x = np.asarray(x, f32)
    x0 = np.asarray(orignal_x, f32)
    adj = np.asarray(adj, np.int32)
    cst, wa1, wa2 = _consts_full_np(W, a)
    wa1_f = wa1.astype(f32)
    wa2_f = wa2.astype(f32)
    n = x.shape[0]
    assert n <= RPC_REAL * ncores

    in_maps = []
    for c in range(ncores):
        lo = c * RPC_REAL
        hi = min((c + 1) * RPC_REAL, n)
        rows = hi - lo
        xc = x[lo:hi]
        x0c = x0[lo:hi]
        adjc = adj[lo:hi]
        if rows < rpc:
            pad = rpc - rows
            xc = np.concatenate([xc, np.zeros((pad, K, F), f32)])
            x0c = np.concatenate([x0c, np.zeros((pad, F), f32)])
            adjc = np.concatenate([adjc, np.zeros((pad, K), np.int32)])
        # per-tile packs [50, 128, *]
        xp = (xc * wa2_f[None, None, :]).astype(BF16)
        xp50 = xp.reshape(ntiles, 16, 8, K, F).transpose(
            0, 2, 3, 1, 4).reshape(ntiles, 128, BPT * F)
        x0w1 = (x0c * wa1_f[None, :]).astype(BF16).reshape(ntiles, 128, F)
        x0p = (x0c * wa2_f[None, :]).astype(BF16).reshape(ntiles, 128, F)
        adj50 = ((adjc - 1.0) * 1000.0).astype(BF16).reshape(
            ntiles, 16, 8, K).transpose(0, 2, 3, 1).reshape(ntiles, 128, K)
        xdev = np.empty((NPAIRS, 128, XC2), BF16)
        xdev[:, :, 0:2048] = xp50[0::2]
        xdev[:, :, 2048:4096] = xp50[1::2]
        xdev[:, :, OFF_X0W1:OFF_X0W1 + F] = x0w1[0::2]
        xdev[:, :, OFF_X0W1 + F:OFF_X0W1 + 2 * F] = x0w1[1::2]
        xdev[:, :, OFF_X0P:OFF_X0P + F] = x0p[0::2]
        xdev[:, :, OFF_X0P + F:OFF_X0P + 2 * F] = x0p[1::2]
        xdev[:, :, OFF_ADJ:OFF_ADJ + K] = adj50[0::2]
        xdev[:, :, OFF_ADJ + K:OFF_ADJ + 2 * K] = adj50[1::2]
        in_maps.append({"xd": xdev, "cst": cst})
    return in_maps


_LAST_RESULTS = None


def kernel(orignal_x, x, adj, W, a):
    import os
    os.environ.setdefault("JAX_PLATFORMS", "")
    from concourse.bass_utils import run_bass_kernel_spmd

    global _LAST_RESULTS
    nc = _get_nc()
    in_maps = _shard_inputs(orignal_x, x, adj, W, a)
    res = run_bass_kernel_spmd(nc, in_maps, list(range(NCORES)))
    _LAST_RESULTS = res
    y = np.concatenate(
        [np.asarray(r["yd"]).astype(np.float32).reshape(
            TILE * NTILES, F)[:RPC_REAL]
         for r in res.results], axis=0)
    return np.ascontiguousarray(y[:N])
